# revision 1
# baseline (speedup 1.0000x reference)
"""Aitchison multi-head attention on 8 trn2 NeuronCores.

Strategy:
- CLR centering is linear -> folded into Wq/Wk + biases on the host (fp64).
- Shard: core c handles batch b=c//4 and 4 heads (feature slice of 256).
  QKV/out projection weights sliced per core; host sums the 4 partial
  output projections per batch and adds bo.
- Device kernel (per core, bf16 matmul operands / fp32 PSUM accum):
    qcT,kcT [256,2048] = W_eff @ x.T (+bias, f on partitions)
    v       [2048,256] = x @ Wv.T + bv (natural layout)
    S.T tiles: per [128,2048] PSUM group, 2 tk-tiles x 2 heads row-packed
    expST = exp(S.T/8) via one ScalarE op per group (overhead amortized)
    PV: both heads col-packed into one PSUM bank (memset + start=False
        sidesteps the bank-wide has_written clear on interleaved chains)
    softmax sums: ones-vector matmuls on the PE (pair-concurrent, cheap)
    normalize via DVE mult with 1/sums broadcast (GPSIMD partition_broadcast)
    out partial = attnT.T @ WoT per 128x512 tile -> DMA out
- Emission is software-pipelined: PV/sums chunks of unit i-1, Wo groups and
  the v/f1 projections are interleaved between score groups so the PE never
  idles long enough (~3.4us) for the HAM to re-throttle its clock; PSUM is
  copied to SBUF before the reciprocal chain so banks free early, and Wo
  groups only drain late in a unit so they never wait on a fresh norm.
"""
import sys
import types

sys.path.insert(0, "/opt/trn_rl_repo")

import numpy as np
import ml_dtypes

import concourse.bass as bass
import concourse.tile as tile
from concourse import bacc, mybir
from concourse.bass_utils import run_bass_kernel_spmd

B, T, E, H, Dh = 2, 2048, 1024, 16, 64
NCORES = 8
HPC = 4            # heads per core
F = HPC * Dh       # 256 features per core
SCALE = 8.0        # sqrt(Dh)
KC = E // 128      # 8 k-chunks in projections
BF = mybir.dt.bfloat16
F32 = mybir.dt.float32
BF_NP = ml_dtypes.bfloat16


def _install_ntff_hook():
    """trace=True under axon needs antenv.axon_hooks, missing in this image."""
    if "antenv.axon_hooks" in sys.modules:
        return
    try:
        from trn_agent_boot.trn_boot import _ntff_profile_via_ctypes

        hook = _ntff_profile_via_ctypes("/opt/axon/libaxon_pjrt.so")
    except Exception:
        hook = None
    mod = types.ModuleType("antenv.axon_hooks")
    mod.get_axon_ntff_profile_hook = lambda: hook
    sys.modules["antenv.axon_hooks"] = mod


def _emit(tc, io):
    nc = tc.nc
    from contextlib import ExitStack

    ctx = ExitStack()
    with ctx:
        const = ctx.enter_context(tc.tile_pool(name="const", bufs=1))
        xpool = ctx.enter_context(tc.tile_pool(name="x", bufs=16))
        qk = ctx.enter_context(tc.tile_pool(name="qk", bufs=1))
        epool = ctx.enter_context(tc.tile_pool(name="exp", bufs=2))
        spool = ctx.enter_context(tc.tile_pool(name="small", bufs=1))
        opool = ctx.enter_context(tc.tile_pool(name="out", bufs=2))
        ps_a = ctx.enter_context(tc.tile_pool(name="psa", bufs=2, space="PSUM"))
        ps_b = ctx.enter_context(tc.tile_pool(name="psb", bufs=4, space="PSUM"))

        def load_w(name):
            ts = []
            for kk in range(KC):
                t = const.tile([128, F], BF, name=f"{name}{kk}", tag=f"{name}{kk}")
                nc.sync.dma_start(t[:], io[name][kk * 128:(kk + 1) * 128, :])
                ts.append(t)
            return ts

        def load_b(name):
            ts = []
            for ft in range(2):
                t = const.tile([128, 1], F32, name=f"{name}{ft}", tag=f"{name}{ft}")
                nc.sync.dma_start(t[:], io[name][ft * 128:(ft + 1) * 128, :])
                ts.append(t)
            return ts

        def load_x(which):
            xc = []
            for kk in range(KC):
                t = xpool.tile([128, T], BF, name="xc", tag="xc")
                nc.sync.dma_start(t[:], io[which][kk * 128:(kk + 1) * 128, :])
                xc.append(t)
            return xc

        # ---- persistent activation tiles ----
        qcT = [qk.tile([128, T], BF, name=f"qcT{ft}", tag=f"qcT{ft}") for ft in range(2)]
        kcT = [qk.tile([128, T], BF, name=f"kcT{ft}", tag=f"kcT{ft}") for ft in range(2)]
        attnT = [qk.tile([128, T], BF, name=f"attnT{ft}", tag=f"attnT{ft}") for ft in range(2)]
        v_pl = [qk.tile([128, F], BF, name=f"vpl{tt}", tag=f"vpl{tt}") for tt in range(16)]

        def proj_qk(wt, bt, dst, xc, ft, tbps=(0, 1, 2, 3)):
            for tbp in tbps:  # 512-wide t groups (psb rotation, no stalls)
                ps = ps_b.tile([128, 512], F32, name="psp", tag="psb")
                tq0 = tbp * 512
                for kk in range(KC):
                    nc.tensor.matmul(
                        ps[:],
                        wt[kk][:, ft * 128:(ft + 1) * 128],
                        xc[kk][:, tq0:tq0 + 512],
                        start=(kk == 0),
                        stop=(kk == KC - 1),
                    )
                nc.vector.tensor_scalar_add(
                    dst[ft][:, tq0:tq0 + 512], ps[:], bt[ft][:]
                )

        def v_tile(xc, wv_t, bv_bc, tt):
            ps = ps_b.tile([128, 256], F32, name="psv", tag="psb")
            for kk in range(KC):
                nc.tensor.matmul(
                    ps[:],
                    xc[kk][:, tt * 128:(tt + 1) * 128],
                    wv_t[kk][:],
                    start=(kk == 0),
                    stop=(kk == KC - 1),
                )
            nc.vector.tensor_tensor(
                v_pl[tt][:], ps[:], bv_bc[:, 0:F], mybir.AluOpType.add
            )

        # --- PV: both heads col-packed in ONE PSUM bank; softmax sums via
        # ones-matmuls into a second bank (rows 0 and 32). All start=False
        # after a data memset (a start=True would clear has_written for the
        # whole bank and break the other head's interleaved chain); the
        # sums matmuls double as PE keep-warm work.
        def pv_start(p, blk):
            pv = ps_b.tile([128, 512], F32, name="pvps", tag="psb")
            sm = ps_b.tile([64, 512], F32, name="smps", tag="psb")
            nc.vector.memset(pv[:], 0.0)
            nc.vector.memset(sm[:], 0.0)
            return (p, blk, pv, sm)

        def pv_chunk(pvst, expt, g):
            p, blk, pv, sm = pvst
            for j in (2 * g, 2 * g + 1):
                for hh in range(2):
                    lh = p * 2 + hh
                    sl = expt[:, (2 * j + hh) * 512:(2 * j + hh + 1) * 512]
                    nc.tensor.matmul(
                        pv[hh * 64:(hh + 1) * 64, :],
                        v_pl[j][:, lh * 64:(lh + 1) * 64],
                        sl,
                        start=False,
                        stop=(j == 15),
                        skip_group_check=True,
                    )
                for hh in range(2):
                    sl = expt[:, (2 * j + hh) * 512:(2 * j + hh + 1) * 512]
                    nc.tensor.matmul(
                        sm[hh * 32:hh * 32 + 1, :],
                        ones_t[:, 0:1],
                        sl,
                        start=False,
                        stop=(j == 15),
                        skip_group_check=True,
                    )

        def pv_finish_copy(pvst):
            p, blk, pv, sm = pvst
            # copy PSUM->SBUF immediately so the banks free early
            smc = spool.tile([33, 512], F32, name="smc", tag="smc")
            nc.vector.tensor_copy(smc[:], sm[0:33, :])
            pvcs = []
            for hh in range(2):
                pvc = spool.tile([64, 512], F32, name=f"pvc{hh}", tag=f"pvc{hh}")
                nc.vector.tensor_copy(pvc[:], pv[hh * 64:(hh + 1) * 64, :])
                pvcs.append(pvc)
            return (p, blk, pvcs, smc)

        def pv_finish_norm(cp):
            p, blk, pvcs, smc = cp
            tq0 = blk * 512
            rcs, rbs = [], []
            for hh in range(2):
                rc = spool.tile([1, 512], F32, name=f"rc{hh}", tag=f"rc{hh}")
                nc.vector.reciprocal(rc[:], smc[hh * 32:hh * 32 + 1, :])
                rcs.append(rc)
            for hh in range(2):
                rb = spool.tile([64, 512], F32, name=f"rb{hh}", tag=f"rb{hh}")
                nc.gpsimd.partition_broadcast(rb[:], rcs[hh][:])
                rbs.append(rb)
            for hh in range(2):
                nc.vector.tensor_tensor(
                    attnT[p][hh * 64:(hh + 1) * 64, tq0:tq0 + 512],
                    pvcs[hh][:],
                    rbs[hh][:],
                    mybir.AluOpType.mult,
                )

        def wo_group(tt):
            for eb in range(2):
                ps = ps_b.tile([128, 512], F32, name="pswo", tag="psb")
                for fc in range(2):
                    nc.tensor.matmul(
                        ps[:],
                        attnT[fc][:, tt * 128:(tt + 1) * 128],
                        wo_t[fc][:, eb * 512:(eb + 1) * 512],
                        start=(fc == 0),
                        stop=(fc == 1),
                    )
                ot = opool.tile([128, 512], F32, name="ot", tag="ot")
                nc.vector.tensor_copy(ot[:], ps[:])
                nc.sync.dma_start(
                    io["out"][tt * 128:(tt + 1) * 128, eb * 512:(eb + 1) * 512],
                    ot[:],
                )

        def warm(n):
            # dummy LDWEIGHTS: keep the PE activity monitor from
            # re-throttling the clock during unavoidable PE slack
            for _ in range(n):
                nc.tensor.ldweights(wo_t[0][:, 0:128])

        def unit_emit(p, blk, prev, wo_pending, mid=None, per_g=None,
                      pv_after=False, warm_n=0):
            """Scores+exp of (p, blk) in 8 groups ([128,2048] PSUM = 2
            tk-tiles x 2 heads row-packed; one 2048-wide exp op per group).
            PV/sums chunks of `prev` = (pvst, expt) and pending Wo groups
            are interleaved as ACT-independent PE work. Returns
            (expt, copies-of-prev-PV)."""
            expt = epool.tile([128, 32 * 512], BF, name="exp", tag="exp")
            tq0 = blk * 512
            pvst = prev[0] if prev is not None else None
            for g in range(8):  # groups of 2 tk tiles x 2 heads
                if pvst is not None and not pv_after:
                    pv_chunk(pvst, prev[1], g)
                # [128,1024] PSUM per tk tile (2 banks) so two groups
                # double-buffer: ACT exps tile g while PE fills g+1
                for j2 in range(2):
                    tk = g * 2 + j2
                    ps = ps_a.tile([128, 1024], F32, name="psa", tag="psa")
                    for hh in range(2):
                        pp = hh * 64
                        nc.tensor.matmul(
                            ps[:, hh * 512:(hh + 1) * 512],
                            kcT[p][pp:pp + 64, tk * 128:(tk + 1) * 128],
                            qcT[p][pp:pp + 64, tq0:tq0 + 512],
                            start=True,
                            stop=True,
                        )
                    nc.scalar.activation(
                        expt[:, tk * 1024:(tk + 1) * 1024],
                        ps[:],
                        mybir.ActivationFunctionType.Exp,
                        scale=1.0 / SCALE,
                    )
                if per_g is not None:
                    per_g(g)
                if pvst is not None and pv_after:
                    pv_chunk(pvst, prev[1], g)
                if g >= 4 and wo_pending:
                    wo_group(wo_pending.pop(0))
                if warm_n:
                    warm(warm_n)
                if mid is not None and g in mid:
                    mid[g]()
            cp = pv_finish_copy(pvst) if pvst is not None else None
            return expt, cp

        # ================= emission schedule =================
        wq_t = load_w("wqT")
        bq_t = load_b("bq")
        xq = load_x("xqT")
        wk_t = load_w("wkT")
        bk_t = load_b("bk")
        xk = load_x("xkT")
        ones_t = const.tile([128, 1], BF, name="ones", tag="ones")
        nc.gpsimd.memset(ones_t[:], 1.0)
        proj_qk(wq_t, bq_t, qcT, xq, 0)
        proj_qk(wk_t, bk_t, kcT, xk, 0, tbps=(0, 1))

        wv_t = load_w("wvT")
        xv = load_x("xvT")
        bv_row = const.tile([1, F], F32, name="bvrow", tag="bvrow")
        nc.sync.dma_start(bv_row[:], io["bv"][:])
        bv_bc = const.tile([128, F], F32, name="bvbc", tag="bvbc")
        nc.gpsimd.partition_broadcast(bv_bc[:], bv_row[:])
        wo_t = []
        for fc in range(2):
            t = const.tile([128, E], BF, name=f"woT{fc}", tag=f"woT{fc}")
            nc.sync.dma_start(t[:], io["woT"][fc * 128:(fc + 1) * 128, :])
            wo_t.append(t)

        wo_pending = []

        mid00 = {
            3: lambda: (proj_qk(wk_t, bk_t, kcT, xk, 0, tbps=(2, 3)),
                        proj_qk(wq_t, bq_t, qcT, xq, 1)),
        }
        expt_prev, _ = unit_emit(0, 0, None, wo_pending, mid=mid00)
        pvst_prev = pv_start(0, 0)

        units = [(0, 1), (0, 2), (0, 3), (1, 0), (1, 1), (1, 2), (1, 3)]
        for ui, (p, blk) in enumerate(units):
            if ui == 0:
                # v-projection tiles interleaved dependency-aligned: PV chunk
                # g needs v_pl[2g], v_pl[2g+1], emitted just before it
                expt, cp = unit_emit(
                    p, blk, (pvst_prev, expt_prev), wo_pending,
                    per_g=lambda g: (v_tile(xv, wv_t, bv_bc, 2 * g),
                                     v_tile(xv, wv_t, bv_bc, 2 * g + 1)),
                    pv_after=True,
                )
            else:
                mid = None
                if ui == 1:
                    mid = {3: lambda: proj_qk(wk_t, bk_t, kcT, xk, 1)}
                expt, cp = unit_emit(p, blk, (pvst_prev, expt_prev),
                                     wo_pending, mid=mid)
            pvst_prev = pv_start(p, blk)
            if cp is not None:
                pv_finish_norm(cp)
                if cp[0] == 1:
                    wo_pending.extend(range(cp[1] * 4, cp[1] * 4 + 4))
            expt_prev = expt
        prev_p, prev_blk = p, blk
        # drain: PV of the last unit, with leftover Wo interleaved.
        # Sums matmuls first so the reciprocal chain overlaps the PV drain.
        _, _, dpv, dsm = pvst_prev
        for j in range(16):
            for hh in range(2):
                sl = expt_prev[:, (2 * j + hh) * 512:(2 * j + hh + 1) * 512]
                nc.tensor.matmul(
                    dsm[hh * 32:hh * 32 + 1, :], ones_t[:, 0:1], sl,
                    start=False, stop=(j == 15), skip_group_check=True,
                )
        for g in range(8):
            for j in (2 * g, 2 * g + 1):
                for hh in range(2):
                    lh = prev_p * 2 + hh
                    sl = expt_prev[:, (2 * j + hh) * 512:(2 * j + hh + 1) * 512]
                    nc.tensor.matmul(
                        dpv[hh * 64:(hh + 1) * 64, :],
                        v_pl[j][:, lh * 64:(lh + 1) * 64],
                        sl,
                        start=False, stop=(j == 15), skip_group_check=True,
                    )
            if g == 0:
                # sums are complete: run the reciprocal chain on DVE/GPSIMD
                # while the PE drains the PV chunks, so the final Wo groups
                # are not gated by it
                dsmc = spool.tile([33, 512], F32, name="dsmc", tag="smc")
                nc.vector.tensor_copy(dsmc[:], dsm[0:33, :])
                drbs = []
                for hh in range(2):
                    rc = spool.tile([1, 512], F32, name=f"drc{hh}", tag=f"rc{hh}")
                    nc.vector.reciprocal(rc[:], dsmc[hh * 32:hh * 32 + 1, :])
                    rb = spool.tile([64, 512], F32, name=f"drb{hh}", tag=f"rb{hh}")
                    nc.gpsimd.partition_broadcast(rb[:], rc[:])
                    drbs.append(rb)
            if g >= 4 and wo_pending:
                wo_group(wo_pending.pop(0))
        tq0 = prev_blk * 512
        for hh in range(2):
            pvc = spool.tile([64, 512], F32, name=f"dpvc{hh}", tag=f"pvc{hh}")
            nc.vector.tensor_copy(pvc[:], dpv[hh * 64:(hh + 1) * 64, :])
            nc.vector.tensor_tensor(
                attnT[prev_p][hh * 64:(hh + 1) * 64, tq0:tq0 + 512],
                pvc[:],
                drbs[hh][:],
                mybir.AluOpType.mult,
            )
        for tt in range(prev_blk * 4, prev_blk * 4 + 4):
            wo_pending.append(tt)
        for tt in wo_pending:
            wo_group(tt)

        if io["debug"]:
            for ft in range(2):
                nc.sync.dma_start(io["d_qcT"][ft * 128:(ft + 1) * 128, :], qcT[ft][:])
                nc.sync.dma_start(io["d_kcT"][ft * 128:(ft + 1) * 128, :], kcT[ft][:])
                nc.sync.dma_start(io["d_attnT"][ft * 128:(ft + 1) * 128, :], attnT[ft][:])
            for tt in range(16):
                nc.sync.dma_start(io["d_vaug"][tt * 128:(tt + 1) * 128, :], v_pl[tt][:])


def _build():
    nc = bacc.Bacc("TRN2", target_bir_lowering=False, debug=False)
    io = {}
    for name, shape, dt in (
        ("xqT", [E, T], BF),
        ("xkT", [E, T], BF),
        ("xvT", [E, T], BF),
        ("wqT", [E, F], BF),
        ("wkT", [E, F], BF),
        ("wvT", [E, F], BF),
        ("woT", [F, E], BF),
        ("bq", [F, 1], F32),
        ("bk", [F, 1], F32),
        ("bv", [1, F], F32),
    ):
        io[name] = nc.dram_tensor(name, shape, dt, kind="ExternalInput").ap()
    io["out"] = nc.dram_tensor("out", [T, E], F32, kind="ExternalOutput").ap()
    import os
    debug = bool(int(os.environ.get("KERNEL_DEBUG", "0")))
    if debug:
        for nm, shape in (("d_qcT", [2 * 128, T]), ("d_kcT", [2 * 128, T]),
                          ("d_attnT", [2 * 128, T]), ("d_vaug", [16 * 128, F])):
            io[nm] = nc.dram_tensor(nm, shape, BF, kind="ExternalOutput").ap()
    io["debug"] = debug
    with tile.TileContext(nc) as tc:
        _emit(tc, io)
    nc.compile()
    return nc


def _fold_clr(W, b, clr):
    """q_c = q - mean_head(q) + clr  ==  x @ (C W).T + (C b + clr)."""
    W64 = W.astype(np.float64).reshape(H, Dh, E)
    W_eff = W64 - W64.mean(axis=1, keepdims=True)
    b64 = b.astype(np.float64).reshape(H, Dh)
    b_eff = b64 - b64.mean(axis=1, keepdims=True) + clr.astype(np.float64).reshape(H, Dh)
    return W_eff.reshape(E, E), b_eff.reshape(E)


_NC_CACHE = None


def kernel(**inputs):
    global _NC_CACHE
    query = np.asarray(inputs["query"], np.float32)
    key = np.asarray(inputs["key"], np.float32)
    value = np.asarray(inputs["value"], np.float32)
    mask = np.asarray(inputs["key_padding_mask"])
    Wq, bq = np.asarray(inputs["Wq"], np.float32), np.asarray(inputs["bq"], np.float32)
    Wk, bk = np.asarray(inputs["Wk"], np.float32), np.asarray(inputs["bk"], np.float32)
    Wv, bv = np.asarray(inputs["Wv"], np.float32), np.asarray(inputs["bv"], np.float32)
    Wo, bo = np.asarray(inputs["Wo"], np.float32), np.asarray(inputs["bo"], np.float32)
    cq = np.asarray(inputs["clr_bias_q"], np.float32)
    ck = np.asarray(inputs["clr_bias_k"], np.float32)
    assert not mask.any(), "kernel assumes empty key_padding_mask"

    Wq_eff, bq_eff = _fold_clr(Wq, bq, cq)
    Wk_eff, bk_eff = _fold_clr(Wk, bk, ck)

    def bf(x):
        return np.ascontiguousarray(x.astype(np.float32)).astype(BF_NP)

    in_maps = []
    for c in range(NCORES):
        b = c // 4
        fs = (c % 4) * F
        m = {
            "xqT": bf(query[b].T),
            "xkT": bf(key[b].T),
            "xvT": bf(value[b].T),
            "wqT": bf(Wq_eff[fs:fs + F].T),
            "wkT": bf(Wk_eff[fs:fs + F].T),
            "wvT": bf(Wv[fs:fs + F].T),
            "woT": bf(Wo[:, fs:fs + F].T),
            "bq": np.ascontiguousarray(bq_eff[fs:fs + F, None], dtype=np.float32),
            "bk": np.ascontiguousarray(bk_eff[fs:fs + F, None], dtype=np.float32),
            "bv": np.ascontiguousarray(bv[None, fs:fs + F], dtype=np.float32),
        }
        in_maps.append(m)

    if _NC_CACHE is None:
        _NC_CACHE = _build()
    nc = _NC_CACHE

    import os

    trace = bool(int(os.environ.get("KERNEL_TRACE", "0")))
    if trace:
        _install_ntff_hook()
    res = None
    last_exc = None
    out = None
    for attempt in range(4):
        try:
            res = run_bass_kernel_spmd(
                nc, in_maps, core_ids=list(range(NCORES)), trace=trace
            )
        except Exception as e:  # transient NRT_EXEC_UNIT_UNRECOVERABLE etc.
            last_exc = e
            import time

            time.sleep(2.0)
            continue
        out = np.zeros((B, T, E), np.float32)
        for c in range(NCORES):
            out[c // 4] += res.results[c]["out"]
        if np.isfinite(out).all():
            break
        out = None  # rare transient corruption: retry
    if out is None:
        if last_exc is not None and res is None:
            raise last_exc
        raise RuntimeError("kernel produced non-finite output on all attempts")
    kernel.last_results = res
    out += bo[None, None, :].astype(np.float32)
    return out

